# revision 12
# baseline (speedup 1.0000x reference)
"""Trainium2 Bass kernel for batched two-layer-MLP attention.

Reference semantics (per batch b):
    x  = sequence[:, b, :]                        # [S, D]
    K  = tanh(tanh(x @ Kw1.T) @ Kw2.T)
    Q  = tanh(tanh(x @ Qw1.T) @ Qw2.T)
    W  = softmax(K @ Q.T / sqrt(D), axis=-1)      # [S, S]
    out[:, b, :] = W @ x

Sharding: data-parallel over batch (B=8 -> 8 NeuronCores), weights replicated.
Compute in bf16 on the TensorEngine (fp32 PSUM accumulation); softmax in fp32.

Layout strategy per core:
  - xt = x.T  [D, S]  (bf16, host-pretransposed)  -> MLP moving operand
  - weights pre-transposed to [d_in, d_out] so they serve directly as lhsT
  - MLP outputs stay transposed: Kt, Qt in [D, S]
  - scores SC[s, t] = sum_d Kt[d,s] * Qt[d,t]: lhsT=Kt tile, rhs=Qt -> natural
  - softmax along free axis (t); exp's accum_out gives the row sums for free
  - exp(SC) tiles are PE-transposed (bf16, 1 cyc/row) to serve as lhsT of
    attended = Wt.T @ x with rhs = xn [S, D] (bf16, natural layout)
  - 1/rowsum is folded into the PSUM->SBUF copy of the output (per-partition
    activation scale), so the big W matrix is never normalized.
"""

import numpy as np
import ml_dtypes

import concourse.bass as bass
import concourse.bacc as bacc
import concourse.tile as tile
from concourse import mybir

from concourse.bass_utils import run_bass_kernel_spmd

P = 128          # partitions
S = 2048         # sequence length
D = 1024         # model dim
B = 8            # batch (one per core)
ST = S // P      # 16 s-tiles
DT = D // P      # 8 d-tiles
NF = 512         # psum free width (one bank of fp32)
SN = S // NF     # 4 score free-chunks
DN = D // NF     # 2 output free-chunks
BF = mybir.dt.bfloat16
F32 = mybir.dt.float32
SCALE = 1.0 / np.sqrt(np.float32(D))

AX = mybir.AxisListType.X
AF = mybir.ActivationFunctionType


def build_nc():
    nc = bacc.Bacc("TRN2", target_bir_lowering=False)

    xt_d = nc.dram_tensor("xt", [D, S], BF, kind="ExternalInput")
    xn_d = nc.dram_tensor("xn", [S, D], BF, kind="ExternalInput")
    wk1_d = nc.dram_tensor("wk1", [D, D], BF, kind="ExternalInput")
    wk2_d = nc.dram_tensor("wk2", [D, D], BF, kind="ExternalInput")
    wq1_d = nc.dram_tensor("wq1", [D, D], BF, kind="ExternalInput")
    wq2_d = nc.dram_tensor("wq2", [D, D], BF, kind="ExternalInput")
    out_d = nc.dram_tensor("out", [S, D], F32, kind="ExternalOutput")

    from contextlib import ExitStack

    with tile.TileContext(nc) as tc, ExitStack() as ctx:
        # ---- persistent SBUF arrays (live across both phases) ----
        pers = ctx.enter_context(tc.tile_pool(name="pers", bufs=1))
        xn_sb = pers.tile([P, ST, D], BF)     # x normal: [t-part, t-tile, d]
        kt_sb = pers.tile([P, DT, S], BF)     # K.T: [d-part, d-tile, s]
        qt_sb = pers.tile([P, DT, S], BF)     # Q.T

        # ---- phase A: the four MLP layers ----
        with tc.tile_pool(name="phase_a", bufs=1) as pa, \
             tc.tile_pool(name="wpool", bufs=2) as wp, \
             tc.tile_pool(name="psum_mlp", bufs=4, space="PSUM") as pm:
            xt_sb = pa.tile([P, DT, S], BF)   # x.T: [d-part, d-tile, s]
            h1_sb = pa.tile([P, DT, S], BF)   # hidden activations (reused K/Q)

            def mlp_layer(src, w_dram, dst, xdma=None, first=False):
                # dst[j, s] = tanh(sum_k w[k, j].T @ src[k, s]) ; all transposed layout
                w_sb = wp.tile([P, DT, D], BF, tag="w")
                if first:
                    # interleave fine-grained w/xt slices so the (j0,n0) psum
                    # group's inputs land first and the PE starts ~3.5us in
                    for k in range(DT):  # w j0-block
                        nc.sync.dma_start(out=w_sb[:, k, 0:P],
                                          in_=w_dram[k * P:(k + 1) * P, 0:P])
                    for n in range(SN):
                        for k in range(DT):  # xt n-chunk
                            nc.sync.dma_start(
                                out=src[:, k, n * NF:(n + 1) * NF],
                                in_=xt_d[k * P:(k + 1) * P, n * NF:(n + 1) * NF])
                        j0, j1 = [(P, 2 * P), (2 * P, 4 * P),
                                  (4 * P, 6 * P), (6 * P, D)][n]
                        for k in range(DT):  # next w j-blocks
                            nc.sync.dma_start(out=w_sb[:, k, j0:j1],
                                              in_=w_dram[k * P:(k + 1) * P, j0:j1])
                else:
                    for k in range(DT):
                        nc.sync.dma_start(out=w_sb[:, k, :],
                                          in_=w_dram[k * P:(k + 1) * P, :])
                if xdma is not None:
                    xdma()
                for j in range(DT):
                    for n in range(SN):
                        ps = pm.tile([P, NF], F32, tag="mlp")
                        for k in range(DT):
                            nc.tensor.matmul(
                                ps,
                                w_sb[:, k, j * P:(j + 1) * P],
                                src[:, k, n * NF:(n + 1) * NF],
                                start=(k == 0),
                                stop=(k == DT - 1),
                            )
                        nc.scalar.activation(
                            out=dst[:, j, n * NF:(n + 1) * NF], in_=ps, func=AF.Tanh
                        )

            def load_xn():
                for t in range(ST):
                    nc.sync.dma_start(out=xn_sb[:, t, :], in_=xn_d[t * P:(t + 1) * P, :])

            mlp_layer(xt_sb, wk1_d, h1_sb, first=True)
            mlp_layer(h1_sb, wk2_d, kt_sb)
            mlp_layer(xt_sb, wq1_d, h1_sb, xdma=load_xn)
            mlp_layer(h1_sb, wq2_d, qt_sb)

        # ---- phase B: scores -> softmax -> transpose -> attended ----
        with tc.tile_pool(name="wexp", bufs=2) as wexp_pool, \
             tc.tile_pool(name="wtT", bufs=2) as wtT_pool, \
             tc.tile_pool(name="sums", bufs=4) as sums_pool, \
             tc.tile_pool(name="outst", bufs=2) as out_pool, \
             tc.tile_pool(name="psum_sc", bufs=3, space="PSUM") as psc, \
             tc.tile_pool(name="psum_at", bufs=3, space="PSUM") as pat:

            def scores_softmax_transpose(i):
                """Row-block i of exp(scores) plus its reciprocal row sums,
                transposed into lhsT layout for the attended matmul."""
                wexp = wexp_pool.tile([P, S], BF, tag="wexp")
                sums = sums_pool.tile([P, SN], F32, tag="sums")
                for n in range(SN):
                    ps = psc.tile([P, NF], F32, tag="sc")
                    for k in range(DT):
                        nc.tensor.matmul(
                            ps,
                            kt_sb[:, k, i * P:(i + 1) * P],
                            qt_sb[:, k, n * NF:(n + 1) * NF],
                            start=(k == 0),
                            stop=(k == DT - 1),
                        )
                    # scores are bounded (|sc/32| < ~3): exp without max-shift
                    nc.scalar.activation(
                        out=wexp[:, n * NF:(n + 1) * NF],
                        in_=ps,
                        func=AF.Exp,
                        scale=float(SCALE),
                        accum_out=sums[:, n:n + 1],
                    )
                rcp = sums_pool.tile([P, 1], F32, tag="rcp")
                nc.vector.reduce_sum(rcp, sums, axis=AX)
                nc.vector.reciprocal(rcp, rcp)
                # one xbar transpose of the whole row-block:
                #   wtT[p, t, c] = wexp[c, t*128 + p]
                wtT = wtT_pool.tile([P, ST, P], BF, tag="wtT")
                nc.scalar.dma_start_transpose(out=wtT, in_=wexp)
                return wtT, rcp

            def attended(i, wtT, rcp):
                outst = out_pool.tile([P, D], F32, tag="outst")
                for n in range(DN):
                    ps = pat.tile([P, NF], F32, tag="at")
                    for t in range(ST):
                        nc.tensor.matmul(
                            ps,
                            wtT[:, t, :],
                            xn_sb[:, t, n * NF:(n + 1) * NF],
                            start=(t == 0),
                            stop=(t == ST - 1),
                        )
                    # fold the softmax normalization into the PSUM->SBUF copy
                    nc.scalar.mul(outst[:, n * NF:(n + 1) * NF], ps, rcp)
                    nc.sync.dma_start(
                        out=out_d[i * P:(i + 1) * P, n * NF:(n + 1) * NF],
                        in_=outst[:, n * NF:(n + 1) * NF],
                    )

            # software-pipelined: attended(i-1) is emitted after scores(i) so
            # the PE never waits on the DVE transpose copies
            prev = None
            for i in range(ST):
                cur = scores_softmax_transpose(i)
                if prev is not None:
                    attended(i - 1, *prev)
                prev = cur
            attended(ST - 1, *prev)

    nc.compile()
    return nc


_NC = None


def _get_nc():
    global _NC
    if _NC is None:
        _NC = build_nc()
    return _NC


def kernel(sequence, Kw1, Kw2, Qw1, Qw2):
    nc = _get_nc()
    bf16 = ml_dtypes.bfloat16

    seq = np.ascontiguousarray(np.transpose(np.asarray(sequence), (1, 0, 2)))  # [B, S, D]
    wk1 = np.ascontiguousarray(np.asarray(Kw1).T).astype(bf16)  # [d_in, d_out]
    wk2 = np.ascontiguousarray(np.asarray(Kw2).T).astype(bf16)
    wq1 = np.ascontiguousarray(np.asarray(Qw1).T).astype(bf16)
    wq2 = np.ascontiguousarray(np.asarray(Qw2).T).astype(bf16)

    in_maps = []
    for b in range(B):
        xb = seq[b]
        in_maps.append({
            "xn": xb.astype(bf16),
            "xt": np.ascontiguousarray(xb.T).astype(bf16),
            "wk1": wk1, "wk2": wk2, "wq1": wq1, "wq2": wq2,
        })

    res = run_bass_kernel_spmd(nc, in_maps, core_ids=list(range(B)))
    out = np.stack([res.results[b]["out"] for b in range(B)], axis=1)
    return out.astype(np.float32)


# revision 13
# speedup vs baseline: 1.0231x; 1.0231x over previous
"""Trainium2 Bass kernel for batched two-layer-MLP attention.

Reference semantics (per batch b):
    x  = sequence[:, b, :]                        # [S, D]
    K  = tanh(tanh(x @ Kw1.T) @ Kw2.T)
    Q  = tanh(tanh(x @ Qw1.T) @ Qw2.T)
    W  = softmax(K @ Q.T / sqrt(D), axis=-1)      # [S, S]
    out[:, b, :] = W @ x

Sharding: data-parallel over batch (B=8 -> 8 NeuronCores), weights replicated.
Compute in bf16 on the TensorEngine (fp32 PSUM accumulation); softmax in fp32.

Layout strategy per core:
  - xt = x.T  [D, S]  (bf16, host-pretransposed)  -> MLP moving operand
  - weights pre-transposed to [d_in, d_out] so they serve directly as lhsT
  - MLP outputs stay transposed: Kt, Qt in [D, S]
  - scores SC[s, t] = sum_d Kt[d,s] * Qt[d,t]: lhsT=Kt tile, rhs=Qt -> natural
  - softmax along free axis (t); exp's accum_out gives the row sums for free
  - exp(SC) tiles are PE-transposed (bf16, 1 cyc/row) to serve as lhsT of
    attended = Wt.T @ x with rhs = xn [S, D] (bf16, natural layout)
  - 1/rowsum is folded into the PSUM->SBUF copy of the output (per-partition
    activation scale), so the big W matrix is never normalized.
"""

import numpy as np
import ml_dtypes

import concourse.bass as bass
import concourse.bacc as bacc
import concourse.tile as tile
from concourse import mybir

from concourse.bass_utils import run_bass_kernel_spmd

P = 128          # partitions
S = 2048         # sequence length
D = 1024         # model dim
B = 8            # batch (one per core)
ST = S // P      # 16 s-tiles
DT = D // P      # 8 d-tiles
NF = 512         # psum free width (one bank of fp32)
SN = S // NF     # 4 score free-chunks
DN = D // NF     # 2 output free-chunks
BF = mybir.dt.bfloat16
F32 = mybir.dt.float32
SCALE = 1.0 / np.sqrt(np.float32(D))

AX = mybir.AxisListType.X
AF = mybir.ActivationFunctionType


def build_nc():
    nc = bacc.Bacc("TRN2", target_bir_lowering=False)

    xt_d = nc.dram_tensor("xt", [D, S], BF, kind="ExternalInput")
    xn_d = nc.dram_tensor("xn", [S, D], BF, kind="ExternalInput")
    wk1_d = nc.dram_tensor("wk1", [D, D], BF, kind="ExternalInput")
    wk2_d = nc.dram_tensor("wk2", [D, D], BF, kind="ExternalInput")
    wq1_d = nc.dram_tensor("wq1", [D, D], BF, kind="ExternalInput")
    wq2_d = nc.dram_tensor("wq2", [D, D], BF, kind="ExternalInput")
    out_d = nc.dram_tensor("out", [S, D], F32, kind="ExternalOutput")

    from contextlib import ExitStack

    with tile.TileContext(nc) as tc, ExitStack() as ctx:
        # ---- persistent SBUF arrays (live across both phases) ----
        pers = ctx.enter_context(tc.tile_pool(name="pers", bufs=1))
        xn_sb = pers.tile([P, ST, D], BF)     # x normal: [t-part, t-tile, d]
        kt_sb = pers.tile([P, DT, S], BF)     # K.T: [d-part, d-tile, s]
        qt_sb = pers.tile([P, DT, S], BF)     # Q.T

        # ---- phase A: the four MLP layers ----
        with tc.tile_pool(name="phase_a", bufs=1) as pa, \
             tc.tile_pool(name="wpool", bufs=2) as wp, \
             tc.tile_pool(name="psum_mlp", bufs=4, space="PSUM") as pm:
            xt_sb = pa.tile([P, DT, S], BF)   # x.T: [d-part, d-tile, s]
            h1_sb = pa.tile([P, DT, S], BF)   # hidden activations (reused K/Q)

            def mlp_layer(src, w_dram, dst, xdma=None, first=False):
                # dst[j, s] = tanh(sum_k w[k, j].T @ src[k, s]) ; all transposed layout
                w_sb = wp.tile([P, DT, D], BF, tag="w")
                if first:
                    # j0 weight block + all of xt first (n-sliced so early psum
                    # groups unblock as slices land), then the remaining weight
                    # columns — the PE starts ~4us in instead of ~18us
                    for k in range(DT):  # w j0-block
                        nc.sync.dma_start(out=w_sb[:, k, 0:P],
                                          in_=w_dram[k * P:(k + 1) * P, 0:P])
                    for n in range(SN):
                        for k in range(DT):  # xt n-chunk
                            nc.sync.dma_start(
                                out=src[:, k, n * NF:(n + 1) * NF],
                                in_=xt_d[k * P:(k + 1) * P, n * NF:(n + 1) * NF])
                    for k in range(DT):
                        nc.sync.dma_start(out=w_sb[:, k, P:D],
                                          in_=w_dram[k * P:(k + 1) * P, P:D])
                else:
                    for k in range(DT):
                        nc.sync.dma_start(out=w_sb[:, k, :],
                                          in_=w_dram[k * P:(k + 1) * P, :])
                if xdma is not None:
                    xdma()
                for j in range(DT):
                    for n in range(SN):
                        ps = pm.tile([P, NF], F32, tag="mlp")
                        for k in range(DT):
                            nc.tensor.matmul(
                                ps,
                                w_sb[:, k, j * P:(j + 1) * P],
                                src[:, k, n * NF:(n + 1) * NF],
                                start=(k == 0),
                                stop=(k == DT - 1),
                            )
                        nc.scalar.activation(
                            out=dst[:, j, n * NF:(n + 1) * NF], in_=ps, func=AF.Tanh
                        )

            def load_xn():
                for t in range(ST):
                    nc.sync.dma_start(out=xn_sb[:, t, :], in_=xn_d[t * P:(t + 1) * P, :])

            mlp_layer(xt_sb, wk1_d, h1_sb, first=True)
            mlp_layer(h1_sb, wk2_d, kt_sb)
            mlp_layer(xt_sb, wq1_d, h1_sb, xdma=load_xn)
            mlp_layer(h1_sb, wq2_d, qt_sb)

        # ---- phase B: scores -> softmax -> transpose -> attended ----
        with tc.tile_pool(name="wexp", bufs=2) as wexp_pool, \
             tc.tile_pool(name="wtT", bufs=2) as wtT_pool, \
             tc.tile_pool(name="sums", bufs=4) as sums_pool, \
             tc.tile_pool(name="outst", bufs=2) as out_pool, \
             tc.tile_pool(name="psum_sc", bufs=3, space="PSUM") as psc, \
             tc.tile_pool(name="psum_at", bufs=3, space="PSUM") as pat:

            def scores_softmax_transpose(i):
                """Row-block i of exp(scores) plus its reciprocal row sums,
                transposed into lhsT layout for the attended matmul."""
                wexp = wexp_pool.tile([P, S], BF, tag="wexp")
                sums = sums_pool.tile([P, SN], F32, tag="sums")
                for n in range(SN):
                    ps = psc.tile([P, NF], F32, tag="sc")
                    for k in range(DT):
                        nc.tensor.matmul(
                            ps,
                            kt_sb[:, k, i * P:(i + 1) * P],
                            qt_sb[:, k, n * NF:(n + 1) * NF],
                            start=(k == 0),
                            stop=(k == DT - 1),
                        )
                    # scores are bounded (|sc/32| < ~3): exp without max-shift
                    nc.scalar.activation(
                        out=wexp[:, n * NF:(n + 1) * NF],
                        in_=ps,
                        func=AF.Exp,
                        scale=float(SCALE),
                        accum_out=sums[:, n:n + 1],
                    )
                rcp = sums_pool.tile([P, 1], F32, tag="rcp")
                nc.vector.reduce_sum(rcp, sums, axis=AX)
                nc.vector.reciprocal(rcp, rcp)
                # one xbar transpose of the whole row-block:
                #   wtT[p, t, c] = wexp[c, t*128 + p]
                wtT = wtT_pool.tile([P, ST, P], BF, tag="wtT")
                nc.scalar.dma_start_transpose(out=wtT, in_=wexp)
                return wtT, rcp

            def attended(i, wtT, rcp):
                outst = out_pool.tile([P, D], F32, tag="outst")
                for n in range(DN):
                    ps = pat.tile([P, NF], F32, tag="at")
                    for t in range(ST):
                        nc.tensor.matmul(
                            ps,
                            wtT[:, t, :],
                            xn_sb[:, t, n * NF:(n + 1) * NF],
                            start=(t == 0),
                            stop=(t == ST - 1),
                        )
                    # fold the softmax normalization into the PSUM->SBUF copy
                    nc.scalar.mul(outst[:, n * NF:(n + 1) * NF], ps, rcp)
                    nc.sync.dma_start(
                        out=out_d[i * P:(i + 1) * P, n * NF:(n + 1) * NF],
                        in_=outst[:, n * NF:(n + 1) * NF],
                    )

            # software-pipelined: attended(i-1) is emitted after scores(i) so
            # the PE never waits on the DVE transpose copies
            prev = None
            for i in range(ST):
                cur = scores_softmax_transpose(i)
                if prev is not None:
                    attended(i - 1, *prev)
                prev = cur
            attended(ST - 1, *prev)

    nc.compile()
    return nc


_NC = None


def _get_nc():
    global _NC
    if _NC is None:
        _NC = build_nc()
    return _NC


def kernel(sequence, Kw1, Kw2, Qw1, Qw2):
    nc = _get_nc()
    bf16 = ml_dtypes.bfloat16

    seq = np.ascontiguousarray(np.transpose(np.asarray(sequence), (1, 0, 2)))  # [B, S, D]
    wk1 = np.ascontiguousarray(np.asarray(Kw1).T).astype(bf16)  # [d_in, d_out]
    wk2 = np.ascontiguousarray(np.asarray(Kw2).T).astype(bf16)
    wq1 = np.ascontiguousarray(np.asarray(Qw1).T).astype(bf16)
    wq2 = np.ascontiguousarray(np.asarray(Qw2).T).astype(bf16)

    in_maps = []
    for b in range(B):
        xb = seq[b]
        in_maps.append({
            "xn": xb.astype(bf16),
            "xt": np.ascontiguousarray(xb.T).astype(bf16),
            "wk1": wk1, "wk2": wk2, "wq1": wq1, "wq2": wq2,
        })

    res = run_bass_kernel_spmd(nc, in_maps, core_ids=list(range(B)))
    out = np.stack([res.results[b]["out"] for b in range(B)], axis=1)
    return out.astype(np.float32)


# revision 14
# speedup vs baseline: 1.0533x; 1.0295x over previous
"""Trainium2 Bass kernel for batched two-layer-MLP attention.

Reference semantics (per batch b):
    x  = sequence[:, b, :]                        # [S, D]
    K  = tanh(tanh(x @ Kw1.T) @ Kw2.T)
    Q  = tanh(tanh(x @ Qw1.T) @ Qw2.T)
    W  = softmax(K @ Q.T / sqrt(D), axis=-1)      # [S, S]
    out[:, b, :] = W @ x

Sharding: data-parallel over batch (B=8 -> 8 NeuronCores), weights replicated.
Compute in bf16 on the TensorEngine (fp32 PSUM accumulation); softmax in fp32.

Layout strategy per core:
  - xt = x.T  [D, S]  (bf16, host-pretransposed)  -> MLP moving operand
  - weights pre-transposed to [d_in, d_out] so they serve directly as lhsT
  - MLP outputs stay transposed: Kt, Qt in [D, S]
  - scores SC[s, t] = sum_d Kt[d,s] * Qt[d,t]: lhsT=Kt tile, rhs=Qt -> natural
  - softmax along free axis (t); exp's accum_out gives the row sums for free
  - exp(SC) tiles are PE-transposed (bf16, 1 cyc/row) to serve as lhsT of
    attended = Wt.T @ x with rhs = xn [S, D] (bf16, natural layout)
  - 1/rowsum is folded into the PSUM->SBUF copy of the output (per-partition
    activation scale), so the big W matrix is never normalized.
"""

import numpy as np
import ml_dtypes

import concourse.bass as bass
import concourse.bacc as bacc
import concourse.tile as tile
from concourse import mybir

from concourse.bass_utils import run_bass_kernel_spmd

P = 128          # partitions
S = 2048         # sequence length
D = 1024         # model dim
B = 8            # batch (one per core)
ST = S // P      # 16 s-tiles
DT = D // P      # 8 d-tiles
NF = 512         # psum free width (one bank of fp32)
SN = S // NF     # 4 score free-chunks
DN = D // NF     # 2 output free-chunks
BF = mybir.dt.bfloat16
F32 = mybir.dt.float32
SCALE = 1.0 / np.sqrt(np.float32(D))

AX = mybir.AxisListType.X
AF = mybir.ActivationFunctionType


def build_nc():
    nc = bacc.Bacc("TRN2", target_bir_lowering=False)

    xt_d = nc.dram_tensor("xt", [D, S], BF, kind="ExternalInput")
    xn_d = nc.dram_tensor("xn", [S, D], BF, kind="ExternalInput")
    wk1_d = nc.dram_tensor("wk1", [D, D], BF, kind="ExternalInput")
    wk2_d = nc.dram_tensor("wk2", [D, D], BF, kind="ExternalInput")
    wq1_d = nc.dram_tensor("wq1", [D, D], BF, kind="ExternalInput")
    wq2_d = nc.dram_tensor("wq2", [D, D], BF, kind="ExternalInput")
    out_d = nc.dram_tensor("out", [S, D], F32, kind="ExternalOutput")

    from contextlib import ExitStack

    with tile.TileContext(nc) as tc, ExitStack() as ctx:
        # ---- persistent SBUF arrays (live across both phases) ----
        pers = ctx.enter_context(tc.tile_pool(name="pers", bufs=1))
        xn_sb = pers.tile([P, ST, D], BF)     # x normal: [t-part, t-tile, d]
        kt_sb = pers.tile([P, DT, S], BF)     # K.T: [d-part, d-tile, s]
        qt_sb = pers.tile([P, DT, S], BF)     # Q.T

        # ---- phase A: the four MLP layers ----
        with tc.tile_pool(name="phase_a", bufs=1) as pa, \
             tc.tile_pool(name="wpool", bufs=2) as wp, \
             tc.tile_pool(name="psum_mlp", bufs=4, space="PSUM") as pm:
            xt_sb = pa.tile([P, DT, S], BF)   # x.T: [d-part, d-tile, s]
            h1_sb = pa.tile([P, DT, S], BF)   # hidden activations (reused K/Q)

            def mlp_layer(src, w_dram, dst, xdma=None, first=False):
                # dst[j, s] = tanh(sum_k w[k, j].T @ src[k, s]) ; all transposed layout
                w_sb = wp.tile([P, DT, D], BF, tag="w")
                if first:
                    # all weights first, then xt n-chunk by n-chunk: with the
                    # n-outer loop below, the first psum row only needs
                    # w (2MB) + xt-n0 (1MB), so the PE starts ~7us in instead
                    # of ~18us, and later n-rows never outrun the DMA stream
                    for k in range(DT):
                        nc.sync.dma_start(out=w_sb[:, k, :],
                                          in_=w_dram[k * P:(k + 1) * P, :])
                    for n in range(SN):
                        for k in range(DT):
                            nc.sync.dma_start(
                                out=src[:, k, n * NF:(n + 1) * NF],
                                in_=xt_d[k * P:(k + 1) * P, n * NF:(n + 1) * NF])
                else:
                    for k in range(DT):
                        nc.sync.dma_start(out=w_sb[:, k, :],
                                          in_=w_dram[k * P:(k + 1) * P, :])
                if xdma is not None:
                    xdma()
                loop = ([(j, n) for n in range(SN) for j in range(DT)] if first
                        else [(j, n) for j in range(DT) for n in range(SN)])
                for j, n in loop:
                    ps = pm.tile([P, NF], F32, tag="mlp")
                    for k in range(DT):
                        nc.tensor.matmul(
                            ps,
                            w_sb[:, k, j * P:(j + 1) * P],
                            src[:, k, n * NF:(n + 1) * NF],
                            start=(k == 0),
                            stop=(k == DT - 1),
                        )
                    nc.scalar.activation(
                        out=dst[:, j, n * NF:(n + 1) * NF], in_=ps, func=AF.Tanh
                    )

            def load_xn():
                for t in range(ST):
                    nc.sync.dma_start(out=xn_sb[:, t, :], in_=xn_d[t * P:(t + 1) * P, :])

            mlp_layer(xt_sb, wk1_d, h1_sb, first=True)
            mlp_layer(h1_sb, wk2_d, kt_sb)
            mlp_layer(xt_sb, wq1_d, h1_sb, xdma=load_xn)
            mlp_layer(h1_sb, wq2_d, qt_sb)

        # ---- phase B: scores -> softmax -> transpose -> attended ----
        with tc.tile_pool(name="wexp", bufs=2) as wexp_pool, \
             tc.tile_pool(name="wtT", bufs=2) as wtT_pool, \
             tc.tile_pool(name="sums", bufs=4) as sums_pool, \
             tc.tile_pool(name="outst", bufs=2) as out_pool, \
             tc.tile_pool(name="psum_sc", bufs=3, space="PSUM") as psc, \
             tc.tile_pool(name="psum_at", bufs=3, space="PSUM") as pat:

            def scores_softmax_transpose(i):
                """Row-block i of exp(scores) plus its reciprocal row sums,
                transposed into lhsT layout for the attended matmul."""
                wexp = wexp_pool.tile([P, S], BF, tag="wexp")
                sums = sums_pool.tile([P, SN], F32, tag="sums")
                for n in range(SN):
                    ps = psc.tile([P, NF], F32, tag="sc")
                    for k in range(DT):
                        nc.tensor.matmul(
                            ps,
                            kt_sb[:, k, i * P:(i + 1) * P],
                            qt_sb[:, k, n * NF:(n + 1) * NF],
                            start=(k == 0),
                            stop=(k == DT - 1),
                        )
                    # scores are bounded (|sc/32| < ~3): exp without max-shift
                    nc.scalar.activation(
                        out=wexp[:, n * NF:(n + 1) * NF],
                        in_=ps,
                        func=AF.Exp,
                        scale=float(SCALE),
                        accum_out=sums[:, n:n + 1],
                    )
                rcp = sums_pool.tile([P, 1], F32, tag="rcp")
                nc.vector.reduce_sum(rcp, sums, axis=AX)
                nc.vector.reciprocal(rcp, rcp)
                # one xbar transpose of the whole row-block:
                #   wtT[p, t, c] = wexp[c, t*128 + p]
                wtT = wtT_pool.tile([P, ST, P], BF, tag="wtT")
                nc.scalar.dma_start_transpose(out=wtT, in_=wexp)
                return wtT, rcp

            def attended(i, wtT, rcp):
                outst = out_pool.tile([P, D], F32, tag="outst")
                for n in range(DN):
                    ps = pat.tile([P, NF], F32, tag="at")
                    for t in range(ST):
                        nc.tensor.matmul(
                            ps,
                            wtT[:, t, :],
                            xn_sb[:, t, n * NF:(n + 1) * NF],
                            start=(t == 0),
                            stop=(t == ST - 1),
                        )
                    # fold the softmax normalization into the PSUM->SBUF copy
                    nc.scalar.mul(outst[:, n * NF:(n + 1) * NF], ps, rcp)
                    nc.sync.dma_start(
                        out=out_d[i * P:(i + 1) * P, n * NF:(n + 1) * NF],
                        in_=outst[:, n * NF:(n + 1) * NF],
                    )

            # software-pipelined: attended(i-1) is emitted after scores(i) so
            # the PE never waits on the DVE transpose copies
            prev = None
            for i in range(ST):
                cur = scores_softmax_transpose(i)
                if prev is not None:
                    attended(i - 1, *prev)
                prev = cur
            attended(ST - 1, *prev)

    nc.compile()
    return nc


_NC = None


def _get_nc():
    global _NC
    if _NC is None:
        _NC = build_nc()
    return _NC


def kernel(sequence, Kw1, Kw2, Qw1, Qw2):
    nc = _get_nc()
    bf16 = ml_dtypes.bfloat16

    seq = np.ascontiguousarray(np.transpose(np.asarray(sequence), (1, 0, 2)))  # [B, S, D]
    wk1 = np.ascontiguousarray(np.asarray(Kw1).T).astype(bf16)  # [d_in, d_out]
    wk2 = np.ascontiguousarray(np.asarray(Kw2).T).astype(bf16)
    wq1 = np.ascontiguousarray(np.asarray(Qw1).T).astype(bf16)
    wq2 = np.ascontiguousarray(np.asarray(Qw2).T).astype(bf16)

    in_maps = []
    for b in range(B):
        xb = seq[b]
        in_maps.append({
            "xn": xb.astype(bf16),
            "xt": np.ascontiguousarray(xb.T).astype(bf16),
            "wk1": wk1, "wk2": wk2, "wq1": wq1, "wq2": wq2,
        })

    res = run_bass_kernel_spmd(nc, in_maps, core_ids=list(range(B)))
    out = np.stack([res.results[b]["out"] for b in range(B)], axis=1)
    return out.astype(np.float32)
